# revision 1
# baseline (speedup 1.0000x reference)
"""CrossBlock Trainium2 kernel: 8-core SPMD, zero cross-core collectives.

Math (per batch b):
  qk0/qk1/v0/v1 = proj(x0/x1); sim_h = qk0_h @ qk1_h^T (scaled)
  out0 = ffn(x0, rowsoftmax(sim) @ v1 @ Wo);  out1 = ffn(x1, rowsoftmax(sim^T) @ v0 @ Wo)

Sharding: core c -> (batch c//2, sequence-half c%2), all 4 heads.
The half is materialized by HOST-side column rotation of x0/x1 so the
device program is identical on all cores (pure SPMD, no collectives):
  - pass A computes E = exp(sim[i-all, j in first-1024]) block-rows and
    accumulates m1^T (+ column sums via a ones-augmented V weight, M=65)
  - pass B mirrors it for E^T -> m0^T (+ row sums)
Softmax normalization is applied post-hoc to the small m^T outputs.
FFN runs in transposed layout (features on partitions) so LN stats come
from ones-matmuls and W2 consumes the gelu output with no transposes.
"""

import numpy as np
import ml_dtypes

import concourse.bacc as bacc
import concourse.mybir as mybir
import concourse.tile as tile
from concourse.bass import ds
from concourse.bass_utils import run_bass_kernel_spmd

B, N, D, H, DH = 4, 2048, 256, 4, 64
NH = N // 2  # 1024, the per-core sequence half
SS = float(DH) ** -0.25
EPS = 1e-5
F32 = mybir.dt.float32
BF16 = mybir.dt.bfloat16
AF = mybir.ActivationFunctionType
ALU = mybir.AluOpType
VW = DH + 1  # augmented v width: 64 value cols + 1 ones col


def _build(nc):
    # ---- DRAM I/O (identical program on every core; per-core data differs) ----
    xT = [nc.dram_tensor(f"x{t}T", [2, 128, N], BF16, kind="ExternalInput") for t in (0, 1)]
    xr = [nc.dram_tensor(f"x{t}r", [NH, D], F32, kind="ExternalInput") for t in (0, 1)]
    wqk = nc.dram_tensor("wqk", [2, 128, D], BF16, kind="ExternalInput")
    wv = nc.dram_tensor("wv", [2, 128, D], BF16, kind="ExternalInput")
    wo = nc.dram_tensor("wo", [2, 128, D], BF16, kind="ExternalInput")
    w1 = nc.dram_tensor("w1", [4, 128, 2 * D], BF16, kind="ExternalInput")
    w2 = nc.dram_tensor("w2", [4, 128, D], BF16, kind="ExternalInput")
    bqk = nc.dram_tensor("bqk", [2, 128, 1], F32, kind="ExternalInput")
    bv = nc.dram_tensor("bv", [128, D], F32, kind="ExternalInput")
    bo = nc.dram_tensor("bo", [2, 128, 1], F32, kind="ExternalInput")
    b1 = nc.dram_tensor("b1", [4, 128, 1], F32, kind="ExternalInput")
    lng = nc.dram_tensor("lng", [4, 128, 1], F32, kind="ExternalInput")
    lnb = nc.dram_tensor("lnb", [4, 128, 1], F32, kind="ExternalInput")
    y = [nc.dram_tensor(f"y{t}", [NH, D], F32, kind="ExternalOutput") for t in (0, 1)]
    dscr = nc.dram_tensor("dscr", [4, N], F32)
    dscr2 = nc.dram_tensor("dscr2", [4, NH], BF16)

    with tile.TileContext(nc) as tc:
        with tc.tile_pool(name="per", bufs=1) as per:
            # ---- load weights / inputs ----
            xT_sb = [[per.tile([128, N], BF16, name=f"x{t}T{k}", tag=f"x{t}T{k}") for k in (0, 1)] for t in (0, 1)]
            for t in (0, 1):
                for k in (0, 1):
                    nc.sync.dma_start(out=xT_sb[t][k][:], in_=xT[t][k])
            wqk_sb = [per.tile([128, D], BF16, name=f"wqk{k}", tag=f"wqk{k}") for k in (0, 1)]
            wv_sb = [per.tile([128, D], BF16, name=f"wv{k}", tag=f"wv{k}") for k in (0, 1)]
            wo_sb = [per.tile([128, D], BF16, name=f"wo{k}", tag=f"wo{k}") for k in (0, 1)]
            w1_sb = [per.tile([128, 2 * D], BF16, name=f"w1{k}", tag=f"w1{k}") for k in range(4)]
            w2_sb = [per.tile([128, D], BF16, name=f"w2{k}", tag=f"w2{k}") for k in range(4)]
            for k in (0, 1):
                nc.sync.dma_start(out=wqk_sb[k][:], in_=wqk[k])
                nc.sync.dma_start(out=wv_sb[k][:], in_=wv[k])
                nc.sync.dma_start(out=wo_sb[k][:], in_=wo[k])
            for k in range(4):
                nc.sync.dma_start(out=w1_sb[k][:], in_=w1[k])
                nc.sync.dma_start(out=w2_sb[k][:], in_=w2[k])
            bqk_sb = [per.tile([128, 1], F32, name=f"bqk{k}", tag=f"bqk{k}") for k in (0, 1)]
            bo_sb = [per.tile([128, 1], F32, name=f"bo{k}", tag=f"bo{k}") for k in (0, 1)]
            b1_sb = [per.tile([128, 1], F32, name=f"b1{k}", tag=f"b1{k}") for k in range(4)]
            lng_sb = [per.tile([128, 1], F32, name=f"lng{k}", tag=f"lng{k}") for k in range(4)]
            lnb_sb = [per.tile([128, 1], F32, name=f"lnb{k}", tag=f"lnb{k}") for k in range(4)]
            bv_sb = per.tile([128, D], F32, tag="bv")
            nc.sync.dma_start(out=bv_sb[:], in_=bv[:])
            for k in (0, 1):
                nc.sync.dma_start(out=bqk_sb[k][:], in_=bqk[k])
                nc.sync.dma_start(out=bo_sb[k][:], in_=bo[k])
            for k in range(4):
                nc.sync.dma_start(out=b1_sb[k][:], in_=b1[k])
                nc.sync.dma_start(out=lng_sb[k][:], in_=lng[k])
                nc.sync.dma_start(out=lnb_sb[k][:], in_=lnb[k])
            ones_sb = per.tile([128, 1], BF16, tag="ones")
            nc.vector.memset(ones_sb[:], 1.0)
            eps_sb = per.tile([1, 1], F32, tag="eps")
            nc.vector.memset(eps_sb[:], EPS)

            # ---- projections: qkT = (Wqk*SS)^T x^T + bqk*SS  (dh on partitions) ----
            qkT_sb = [[per.tile([128, N], BF16, name=f"qk{t}T{m}", tag=f"qk{t}T{m}") for m in (0, 1)] for t in (0, 1)]
            with tc.tile_pool(name="pj", bufs=2, space="PSUM") as pj:
              for t in (0, 1):
                for m in (0, 1):
                    ps = pj.tile([128, N], F32)
                    for k in (0, 1):
                        for jc in range(4):
                            nc.tensor.matmul(
                                ps[:, ds(512 * jc, 512)],
                                lhsT=wqk_sb[k][:, ds(128 * m, 128)],
                                rhs=xT_sb[t][k][:, ds(512 * jc, 512)],
                                start=(k == 0), stop=(k == 1),
                            )
                    nc.scalar.activation(qkT_sb[t][m][:], ps[:], AF.Identity, bias=bqk_sb[m][:])

            # ---- projections: v = x @ Wv + bv, augmented with a ones column ----
            # v_sb layout: [128, 16*VW*4? no: [128, 16 blocks * (4 heads * VW)]
            v_sb = [per.tile([128, 16 * 4 * VW], BF16, name=f"v{t}", tag=f"v{t}") for t in (0, 1)]
            with tc.tile_pool(name="pv", bufs=4, space="PSUM") as pv:
                for t in (0, 1):
                    # ones columns, one strided memset: [128, 16, 4, 1]
                    ones_ap = v_sb[t][:, :].rearrange("p (i h c) -> p i h c", i=16, c=VW)[:, :, :, DH:VW]
                    nc.vector.memset(ones_ap, 1.0)
                    for ib in range(16):
                        ps = pv.tile([128, D], F32)
                        for k in (0, 1):
                            nc.tensor.matmul(
                                ps[:],
                                lhsT=xT_sb[t][k][:, ds(128 * ib, 128)],
                                rhs=wv_sb[k][:],
                                start=(k == 0), stop=(k == 1),
                            )
                        dst = v_sb[t][:, ds(ib * 4 * VW, 4 * VW)].rearrange(
                            "p (h c) -> p h c", c=VW)[:, :, 0:DH]
                        src = ps[:, :].rearrange("p (h c) -> p h c", c=DH)
                        bvb = bv_sb[:, :].rearrange("p (h c) -> p h c", c=DH)
                        nc.vector.tensor_tensor(out=dst, in0=src, in1=bvb, op=ALU.add)

            # ---- attention: per head-pair, pass A (E -> m1T) and pass B (E^T -> m0T) ----
            mT_sb = [[per.tile([128, NH], BF16, name=f"m{t}T{p}", tag=f"m{t}T{p}") for p in (0, 1)] for t in (0, 1)]
            # mT_sb[1] <- pass A (m1), mT_sb[0] <- pass B (m0)
            for p in (0, 1):  # head pair: heads (2p, 2p+1)
                for pas in (0, 1):  # 0: E-direction -> m1; 1: E^T-direction -> m0
                    qa = qkT_sb[0] if pas == 0 else qkT_sb[1]   # stationary side
                    qb = qkT_sb[1] if pas == 0 else qkT_sb[0]   # moving side
                    vv = v_sb[0] if pas == 0 else v_sb[1]       # AV weights
                    mdst = mT_sb[1 - pas]
                    with (
                        tc.tile_pool(name="psim", bufs=2, space="PSUM") as psim,
                        tc.tile_pool(name="pm", bufs=1, space="PSUM") as pmp,
                        tc.tile_pool(name="epool", bufs=4) as epool,
                        tc.tile_pool(name="srow", bufs=1) as srow,
                    ):
                        pm = pmp.tile([128, N], F32, name="pm")
                        for ib in range(16):
                            for hh in (0, 1):
                                h = 2 * p + hh
                                sp = psim.tile([128, NH], F32)
                                e = epool.tile([128, NH], BF16)
                                for jc in (0, 1):
                                    nc.tensor.matmul(
                                        sp[:, ds(512 * jc, 512)],
                                        lhsT=qa[p][ds(64 * hh, 64), ds(128 * ib, 128)],
                                        rhs=qb[p][ds(64 * hh, 64), ds(512 * jc, 512)],
                                        start=True, stop=True,
                                    )
                                nc.scalar.activation(e[:], sp[:], AF.Exp)
                                for jc in (0, 1):
                                    nc.tensor.matmul(
                                        pm[0:VW, ds(NH * hh + 512 * jc, 512)],
                                        lhsT=vv[:, ds(ib * 4 * VW + h * VW, VW)],
                                        rhs=e[:, ds(512 * jc, 512)],
                                        start=(ib == 0), stop=(ib == 15),
                                    )
                        # denominators: pm row DH holds col/row sums for both heads
                        lnr = srow.tile([128, N], F32, name="lnr", tag="lnr")
                        nc.scalar.activation(lnr[64:65, :], pm[DH:VW, :], AF.Ln)
                        rcp = srow.tile([128, N], F32, name="rcp", tag="rcp")
                        nc.scalar.activation(rcp[64:65, :], lnr[64:65, :], AF.Exp, scale=-1.0)
                        rep = srow.tile([128, NH], F32, name="rep", tag="rep")
                        stg = srow.tile([128, NH], BF16, name="stg", tag="stg")
                        pdi = 2 * p + pas
                        nc.sync.dma_start(out=dscr[pdi:pdi + 1, :], in_=rcp[64:65, :])
                        for hh in (0, 1):
                            nc.sync.dma_start(
                                out=rep[0:DH, :],
                                in_=dscr[pdi, ds(NH * hh, NH)].unsqueeze(0).broadcast_to((DH, NH)))
                            nc.vector.tensor_tensor(
                                out=stg[0:DH, :],
                                in0=pm[0:DH, ds(NH * hh, NH)],
                                in1=rep[0:DH, :],
                                op=ALU.mult,
                            )
                            nc.sync.dma_start(
                                out=mdst[p][ds(64 * hh, 64), :], in_=stg[0:DH, :])

            # ---- Wo: mprojT = Wo^T @ mT (+bo), per side ----
            mproj_sb = [[per.tile([128, NH], BF16, name=f"mp{t}{m}", tag=f"mp{t}{m}") for m in (0, 1)] for t in (0, 1)]
            with tc.tile_pool(name="pw", bufs=2, space="PSUM") as pw:
                for t in (0, 1):
                    for m in (0, 1):
                        ps = pw.tile([128, NH], F32)
                        for k in (0, 1):
                            for jc in (0, 1):
                                nc.tensor.matmul(
                                    ps[:, ds(512 * jc, 512)],
                                    lhsT=wo_sb[k][:, ds(128 * m, 128)],
                                    rhs=mT_sb[t][k][:, ds(512 * jc, 512)],
                                    start=(k == 0), stop=(k == 1),
                                )
                        nc.scalar.activation(mproj_sb[t][m][:], ps[:], AF.Identity, bias=bo_sb[m][:])

            # ---- FFN (transposed layout), W1 + stats for both sides, then gelu, then W2 ----
            h_sb = [[per.tile([128, NH], BF16, name=f"h{t}{m}", tag=f"h{t}{m}") for m in range(4)] for t in (0, 1)]
            mu_neg = [per.tile([1, NH], BF16, name=f"mu{t}", tag=f"mu{t}") for t in (0, 1)]
            rsig = [per.tile([1, NH], BF16, name=f"rs{t}", tag=f"rs{t}") for t in (0, 1)]
            with (
                tc.tile_pool(name="ph", bufs=2, space="PSUM") as ph,
                tc.tile_pool(name="pst", bufs=1, space="PSUM") as pst,
                tc.tile_pool(name="hsq", bufs=4) as hsqp,
                tc.tile_pool(name="sst", bufs=4) as sstp,
            ):
                for t in (0, 1):
                    ck = [xT_sb[t][0][:, 0:NH], xT_sb[t][1][:, 0:NH],
                          mproj_sb[t][0][:, :], mproj_sb[t][1][:, :]]
                    hsq = [hsqp.tile([128, NH], BF16, name=f"hsq{_}", tag="hsq") for _ in range(4)]
                    for m in range(4):
                        ps = ph.tile([128, NH], F32)
                        for k in range(4):
                            for jc in (0, 1):
                                nc.tensor.matmul(
                                    ps[:, ds(512 * jc, 512)],
                                    lhsT=w1_sb[k][:, ds(128 * m, 128)],
                                    rhs=ck[k][:, ds(512 * jc, 512)],
                                    start=(k == 0), stop=(k == 3),
                                )
                        nc.vector.tensor_scalar(
                            out=h_sb[t][m][:], in0=ps[:],
                            scalar1=b1_sb[m][:], scalar2=None, op0=ALU.add)
                        nc.vector.tensor_tensor(
                            out=hsq[m][:], in0=h_sb[t][m][:], in1=h_sb[t][m][:], op=ALU.mult)
                    sum_ps = pst.tile([1, NH], F32)
                    ssq_ps = pst.tile([1, NH], F32)
                    for k in range(4):
                        for jc in (0, 1):
                            nc.tensor.matmul(
                                sum_ps[:, ds(512 * jc, 512)],
                                lhsT=ones_sb[:], rhs=h_sb[t][k][:, ds(512 * jc, 512)],
                                start=(k == 0), stop=(k == 3),
                            )
                    for k in range(4):
                        for jc in (0, 1):
                            nc.tensor.matmul(
                                ssq_ps[:, ds(512 * jc, 512)],
                                lhsT=ones_sb[:], rhs=hsq[k][:, ds(512 * jc, 512)],
                                start=(k == 0), stop=(k == 3),
                            )
                    # mu_neg = -sum/512 ; var = (ssq - sum^2/512)/512 ; rsig = exp(-0.5*ln(var+eps))
                    nc.vector.tensor_scalar(
                        out=mu_neg[t][:], in0=sum_ps[:],
                        scalar1=-1.0 / 512.0, scalar2=None, op0=ALU.mult)
                    t2 = sstp.tile([1, NH], F32)
                    nc.vector.tensor_tensor(out=t2[:], in0=sum_ps[:], in1=mu_neg[t][:], op=ALU.mult)
                    u = sstp.tile([1, NH], F32)
                    nc.vector.tensor_tensor(out=u[:], in0=ssq_ps[:], in1=t2[:], op=ALU.add)
                    lnv = sstp.tile([1, NH], F32)
                    nc.scalar.activation(lnv[:], u[:], AF.Ln, bias=eps_sb[:], scale=1.0 / 512.0)
                    nc.scalar.activation(rsig[t][:], lnv[:], AF.Exp, scale=-0.5)

                # gelu for both sides (single ACT table switch), then W2 + residual
                t_sb = [[per.tile([128, NH], BF16, name=f"g{t}{m}", tag=f"g{t}{m}") for m in range(4)] for t in (0, 1)]
                murep = [per.tile([128, NH], BF16, name=f"murep{t}", tag=f"murep{t}") for t in (0, 1)]
                rsrep = [per.tile([128, NH], BF16, name=f"rsrep{t}", tag=f"rsrep{t}") for t in (0, 1)]
                for t in (0, 1):
                    nc.sync.dma_start(out=dscr2[t:t + 1, :], in_=mu_neg[t][:])
                    nc.sync.dma_start(out=dscr2[2 + t:3 + t, :], in_=rsig[t][:])
                    nc.sync.dma_start(out=murep[t][:], in_=dscr2[t, :].unsqueeze(0).broadcast_to((128, NH)))
                    nc.sync.dma_start(out=rsrep[t][:], in_=dscr2[2 + t, :].unsqueeze(0).broadcast_to((128, NH)))
                for t in (0, 1):
                    for m in range(4):
                        d1 = sstp.tile([128, NH], BF16, tag="d1")
                        nc.vector.tensor_tensor(
                            out=d1[:], in0=h_sb[t][m][:],
                            in1=murep[t][:], op=ALU.add)
                        d2 = sstp.tile([128, NH], BF16, tag="d2")
                        nc.vector.tensor_tensor(
                            out=d2[:], in0=d1[:],
                            in1=rsrep[t][:], op=ALU.mult)
                        nc.scalar.activation(
                            t_sb[t][m][:], d2[:], AF.Gelu,
                            bias=lnb_sb[m][:], scale=lng_sb[m][:])

            with (
                tc.tile_pool(name="po", bufs=4, space="PSUM") as po,
                tc.tile_pool(name="xrp", bufs=4) as xrp,
                tc.tile_pool(name="outp", bufs=4) as outp,
            ):
                for t in (0, 1):
                    for nb in range(8):
                        ps = po.tile([128, D], F32)
                        for k in range(4):
                            nc.tensor.matmul(
                                ps[:],
                                lhsT=t_sb[t][k][:, ds(128 * nb, 128)],
                                rhs=w2_sb[k][:],
                                start=(k == 0), stop=(k == 3),
                            )
                        xt = xrp.tile([128, D], F32)
                        nc.sync.dma_start(out=xt[:], in_=xr[t][ds(128 * nb, 128), :])
                        ot = outp.tile([128, D], F32)
                        nc.vector.tensor_tensor(out=ot[:], in0=ps[:], in1=xt[:], op=ALU.add)
                        nc.sync.dma_start(out=y[t][ds(128 * nb, 128), :], in_=ot[:])
    return nc


_CACHE = {}


def _get_program():
    if "nc" not in _CACHE:
        nc = bacc.Bacc()
        _build(nc)
        nc.finalize()
        _CACHE["nc"] = nc
    return _CACHE["nc"]


def _bf16(a):
    return np.ascontiguousarray(a.astype(ml_dtypes.bfloat16))


def _f32(a):
    return np.ascontiguousarray(a.astype(np.float32))


def kernel(x0, x1, Wqk, bqk, Wv, bv, Wo, bo, W1, b1, ln_g, ln_b, W2, b2):
    x0, x1 = np.asarray(x0, np.float32), np.asarray(x1, np.float32)
    Wqk = np.asarray(Wqk, np.float32)
    Wv = np.asarray(Wv, np.float32)
    Wo = np.asarray(Wo, np.float32)
    W1 = np.asarray(W1, np.float32)
    W2 = np.asarray(W2, np.float32)
    bqk = np.asarray(bqk, np.float32)
    bv = np.asarray(bv, np.float32)
    bo = np.asarray(bo, np.float32)
    b1 = np.asarray(b1, np.float32)
    b2 = np.asarray(b2, np.float32)
    ln_g = np.asarray(ln_g, np.float32)
    ln_b = np.asarray(ln_b, np.float32)

    shared = {
        "wqk": _bf16((Wqk * SS).reshape(2, 128, D)),
        "wv": _bf16(Wv.reshape(2, 128, D)),
        "wo": _bf16(Wo.reshape(2, 128, D)),
        "w1": _bf16(W1.reshape(4, 128, 2 * D)),
        "w2": _bf16(W2.reshape(4, 128, D)),
        "bqk": _f32((bqk * SS).reshape(2, 128, 1)),
        "bv": _f32(np.broadcast_to(bv.reshape(1, D), (128, D))),
        "bo": _f32(bo.reshape(2, 128, 1)),
        "b1": _f32(b1.reshape(4, 128, 1)),
        "lng": _f32(ln_g.reshape(4, 128, 1)),
        "lnb": _f32(ln_b.reshape(4, 128, 1)),
    }
    in_maps = []
    for c in range(8):
        b, half = c // 2, c % 2
        p0, p1 = x0[b], x1[b]
        if half == 1:
            p0 = np.concatenate([p0[NH:], p0[:NH]], 0)
            p1 = np.concatenate([p1[NH:], p1[:NH]], 0)
        m = dict(shared)
        m["x0T"] = _bf16(p0.T.reshape(2, 128, N))
        m["x1T"] = _bf16(p1.T.reshape(2, 128, N))
        m["x0r"] = _f32(p0[:NH] + b2)
        m["x1r"] = _f32(p1[:NH] + b2)
        in_maps.append(m)

    nc = _get_program()
    res = run_bass_kernel_spmd(nc, in_maps, list(range(8)))
    out0 = np.empty((B, N, D), np.float32)
    out1 = np.empty((B, N, D), np.float32)
    for c in range(8):
        b, half = c // 2, c % 2
        out0[b, half * NH:(half + 1) * NH] = res.results[c]["y0"]
        out1[b, half * NH:(half + 1) * NH] = res.results[c]["y1"]
    return out0, out1



# revision 2
# speedup vs baseline: 8778.6565x; 8778.6565x over previous
"""CrossBlock Trainium2 kernel v3: 8-core SPMD, fp8 DoubleRow attention, fused
softmax denominators, FFN overlapped under the exp stream.

Sharding: core c -> (batch c//2, seq-half c%2) via host token rotation.
Per (pass, head-pair, head): sim fp8-DR (k-tiles = dh halves) -> exp (ACT) ->
AV fp8-DR with v padded to M=128 whose col 64 is ones, so PSUM row 64
accumulates the softmax denominator in the same matmuls. Normalization is
deferred: unnormalized m^T drains to SBUF, denominator rows go to DRAM, one
batched DVE reciprocal per side + broadcast + multiply produce mT fp8.
ACT runs only: Exp (attention), Sqrt (LN rsig), Gelu (tail) = 3 table loads.
Wo / W2 are fp8-DR; W1/stats run under the attention exp stream in 2 spare
PSUM banks. Copies: qkT on DVE, v-scatter + den rows on GpSimd (Pool).
"""

import numpy as np
import ml_dtypes

import concourse.bacc as bacc
import concourse.mybir as mybir
import concourse.tile as tile
from concourse.bass import ds
from concourse.bass_utils import run_bass_kernel_spmd

B, N, D, H, DH = 4, 2048, 256, 4, 64
NH = N // 2
SS = float(DH) ** -0.25
EPS = 1e-5
F32 = mybir.dt.float32
BF16 = mybir.dt.bfloat16
FP8 = mybir.dt.float8e4
AF = mybir.ActivationFunctionType
ALU = mybir.AluOpType
PM = mybir.MatmulPerfMode.DoubleRow
UNITS = [(0, 0), (0, 1), (1, 0), (1, 1)]  # (pass, head-pair); qb side = 1-pass


def _build(nc):
    xTq = [nc.dram_tensor(f"x{t}Tq", [128, 2, N], FP8, kind="ExternalInput") for t in (0, 1)]
    xTh = [nc.dram_tensor(f"x{t}Th", [2, 128, NH], BF16, kind="ExternalInput") for t in (0, 1)]
    wqkp = nc.dram_tensor("wqkp", [2, 128, 2, 128], FP8, kind="ExternalInput")
    bqkp = nc.dram_tensor("bqkp", [2, 128, 1], F32, kind="ExternalInput")
    wv = nc.dram_tensor("wv", [128, 2, D], FP8, kind="ExternalInput")
    bv2 = nc.dram_tensor("bv2", [128, 2, 2, 4, DH], F32, kind="ExternalInput")
    wop = nc.dram_tensor("wop", [128, 2, D], FP8, kind="ExternalInput")
    bo = nc.dram_tensor("bo", [2, 128, 1], F32, kind="ExternalInput")
    w1 = nc.dram_tensor("w1", [4, 128, 2 * D], BF16, kind="ExternalInput")
    w2 = nc.dram_tensor("w2", [4, 128, D], BF16, kind="ExternalInput")
    b1 = nc.dram_tensor("b1", [4, 128, 1], F32, kind="ExternalInput")
    lng = nc.dram_tensor("lng", [4, 128, 1], F32, kind="ExternalInput")
    lnb = nc.dram_tensor("lnb", [4, 128, 1], F32, kind="ExternalInput")
    y = [nc.dram_tensor(f"y{t}", [2, 128, NH], F32, kind="ExternalOutput") for t in (0, 1)]
    dscr = nc.dram_tensor("dscr", [8, NH], F32)
    dscr_r = nc.dram_tensor("dscr_r", [8, NH], F32)
    dscr2 = nc.dram_tensor("dscr2", [4, NH], BF16)

    with tile.TileContext(nc) as tc:
        with (
            tc.tile_pool(name="per", bufs=1) as per,
            tc.tile_pool(name="epool", bufs=9) as epool,
            tc.tile_pool(name="srow", bufs=2) as srow,
            tc.tile_pool(name="sst", bufs=1) as sstp,
            tc.tile_pool(name="st1", bufs=1) as st1p,
            tc.tile_pool(name="hsqp", bufs=1) as hsqp,
        ):
            # ---- loads ----
            wqkp_sb = [per.tile([128, 2, 128], FP8, name=f"wqkp{t}", tag=f"wqkp{t}") for t in (0, 1)]
            bqkp_sb = [per.tile([128, 1], F32, name=f"bqkp{t}", tag=f"bqkp{t}") for t in (0, 1)]
            for t in (0, 1):
                nc.sync.dma_start(out=bqkp_sb[t][:], in_=bqkp[t])
                nc.sync.dma_start(out=wqkp_sb[t][:], in_=wqkp[t])
            wv_sb = per.tile([128, 2, D], FP8, tag="wv")
            nc.sync.dma_start(out=wv_sb[:], in_=wv[:])
            xTq_sb = [per.tile([128, 2, N], FP8, name=f"x{t}Tq", tag=f"x{t}Tq") for t in (0, 1)]
            for t in (0, 1):
                nc.sync.dma_start(out=xTq_sb[t][:], in_=xTq[t][:])
            xTh_sb = [[per.tile([128, NH], BF16, name=f"x{t}Th{k}", tag=f"x{t}Th{k}") for k in (0, 1)] for t in (0, 1)]
            for t in (0, 1):
                for k in (0, 1):
                    nc.sync.dma_start(out=xTh_sb[t][k][:], in_=xTh[t][k])
            bv4_sb = per.tile([128, 2, 2, 4, DH], F32, tag="bv4")
            nc.sync.dma_start(out=bv4_sb[:], in_=bv2[:])
            wop_sb = per.tile([128, 2, D], FP8, tag="wop")
            nc.sync.dma_start(out=wop_sb[:], in_=wop[:])
            bo_sb = [per.tile([128, 1], F32, name=f"bo{m}", tag=f"bo{m}") for m in (0, 1)]
            for m in (0, 1):
                nc.sync.dma_start(out=bo_sb[m][:], in_=bo[m])
            w1_sb = [per.tile([128, 2 * D], BF16, name=f"w1{k}", tag=f"w1{k}") for k in range(4)]
            for k in range(4):
                nc.sync.dma_start(out=w1_sb[k][:], in_=w1[k])
            w2_sb = [per.tile([128, D], BF16, name=f"w2{k}", tag=f"w2{k}") for k in range(4)]
            for k in range(4):
                nc.sync.dma_start(out=w2_sb[k][:], in_=w2[k])
            b1_sb = [per.tile([128, 1], F32, name=f"b1{k}", tag=f"b1{k}") for k in range(4)]
            lng_sb = [per.tile([128, 1], F32, name=f"lng{k}", tag=f"lng{k}") for k in range(4)]
            lnb_sb = [per.tile([128, 1], F32, name=f"lnb{k}", tag=f"lnb{k}") for k in range(4)]
            for k in range(4):
                nc.sync.dma_start(out=b1_sb[k][:], in_=b1[k])
                nc.sync.dma_start(out=lng_sb[k][:], in_=lng[k])
                nc.sync.dma_start(out=lnb_sb[k][:], in_=lnb[k])
            ones_sb = per.tile([128, 1], BF16, tag="ones")
            nc.vector.memset(ones_sb[:], 1.0)

            qkT = [per.tile([128, 2, N], FP8, name=f"qkT{t}", tag=f"qkT{t}") for t in (0, 1)]
            # v: [tok128, bp8, kt2, h4, 128]; col 64 = ones (den), cols 65+ unused
            v_sb = [per.tile([128, 8, 2, 4, 128], FP8, name=f"v{t}", tag=f"v{t}") for t in (0, 1)]
            for s in (0, 1):
                nc.gpsimd.memset(v_sb[s][:, :, :, :, 64:65], 1.0)
            mT = [per.tile([128, 2, NH], FP8, name=f"mT{t}", tag=f"mT{t}") for t in (0, 1)]
            munn = [per.tile([128, NH], BF16, name=f"munn{u}", tag=f"munn{u}") for u in range(4)]
            mproj = [[per.tile([128, NH], BF16, name=f"mp{t}{m}", tag=f"mp{t}{m}") for m in (0, 1)] for t in (0, 1)]
            h_sb = [[per.tile([128, NH], BF16, name=f"h{t}{m}", tag=f"h{m}") for m in range(4)] for t in (0, 1)]
            mu_neg = [per.tile([1, NH], BF16, name=f"mu{t}", tag=f"mu{t}") for t in (0, 1)]
            rsig = [per.tile([1, NH], BF16, name=f"rs{t}", tag=f"rs{t}") for t in (0, 1)]
            rv_t = [per.tile([1, NH], F32, name=f"rv{t}", tag=f"rv{t}") for t in (0, 1)]
            t_sb = [[per.tile([128, NH], BF16, name=f"g{t}{m}", tag=f"g{m}") for m in range(4)] for t in (0, 1)]

            P = {}
            rep_tiles = [None] * 4
            stats_ps = [None, None]
            # ---- emission helpers ----
            def qkproj():
                for s in (0, 1):
                    for half in (0, 1):
                        for t in (0, 1):
                            sp = P["psim"].tile([128, NH], F32)
                            for jj in (0, 1):
                                nc.tensor.matmul(
                                    sp[:, ds(512 * jj, 512)],
                                    lhsT=wqkp_sb[t][:],
                                    rhs=xTq_sb[s][:, :, ds(1024 * half + 512 * jj, 512)],
                                    start=True, stop=True, perf_mode=PM,
                                )
                            if t == 0:
                                nc.scalar.activation(
                                    qkT[s][:, t, ds(NH * half, NH)], sp[:],
                                    AF.Identity, bias=bqkp_sb[t][:])
                            else:
                                nc.vector.tensor_scalar(
                                    out=qkT[s][:, t, ds(NH * half, NH)],
                                    in0=sp[:],
                                    scalar1=bqkp_sb[t][:], scalar2=None, op0=ALU.add)

            def vproj(s):
                for bq in range(4):  # 2 block-pairs per psum tile
                    ps = P["ovl"].tile([128, NH], F32, name="ovl", tag="ovl")
                    for q in range(4):  # 4 token-blocks of 128
                        ib = 4 * bq + q
                        nc.tensor.matmul(
                            ps[:, ds(256 * q, 256)],
                            lhsT=xTq_sb[s][:, :, ds(128 * ib, 128)],
                            rhs=wv_sb[:],
                            start=True, stop=True, perf_mode=PM,
                        )
                    nc.vector.tensor_tensor(
                        out=v_sb[s][:, ds(2 * bq, 2), :, :, 0:DH],
                        in0=ps[:, :].rearrange("p (b t h c) -> p b t h c", b=2, t=2, c=DH),
                        in1=bv4_sb[:],
                        op=ALU.add)

            POLY_PAIRS = ()

            def emit_E(u, b, t2, sp, eb):
                if u > 0 and b in POLY_PAIRS:
                    c = sstp.tile([128, NH], BF16, name="px", tag="px")
                    nc.vector.tensor_scalar(
                        out=c[:], in0=sp[:], scalar1=1.0, scalar2=None, op0=ALU.mult)
                    v = sstp.tile([128, NH], BF16, name="py", tag="py")
                    nc.vector.tensor_scalar(
                        out=v[:], in0=c[:], scalar1=0.5, scalar2=1.0,
                        op0=ALU.mult, op1=ALU.add)
                    w = sstp.tile([128, NH], BF16, name="pw", tag="pz")
                    nc.vector.tensor_tensor(out=w[:], in0=v[:], in1=c[:], op=ALU.mult)
                    nc.vector.tensor_scalar(
                        out=eb[:, t2, :], in0=w[:], scalar1=1.0, scalar2=None, op0=ALU.add)
                else:
                    nc.scalar.activation(eb[:, t2, :], sp[:], AF.Exp)

            def sims_exps(u, hh):
                pas, p = UNITS[u]
                qa, qb = (0, 1) if pas == 0 else (1, 0)
                h = 2 * p + hh
                qa_ap = qkT[qa][ds(32 * h, 32), :, :]
                qb_ap = qkT[qb][ds(32 * h, 32), :, :]
                ebs = []
                for b in range(8):
                    eb = epool.tile([128, 2, NH], FP8, name="eb", tag="eb")
                    ebs.append(eb)
                    for t2 in (0, 1):
                        ib = 2 * b + t2
                        sp = P["psim"].tile([128, NH], F32)
                        for jc in (0, 1):
                            nc.tensor.matmul(
                                sp[:, ds(512 * jc, 512)],
                                lhsT=qa_ap[:, :, ds(128 * ib, 128)],
                                rhs=qb_ap[:, :, ds(512 * jc, 512)],
                                start=True, stop=True, perf_mode=PM,
                                tile_position=(32 * h, 0),
                            )
                        nc.scalar.activation(eb[:, t2, :], sp[:], AF.Exp)
                return ebs

            def av_one(u, hh, b, eb, pm):
                pas, p = UNITS[u]
                qa = 0 if pas == 0 else 1
                h = 2 * p + hh
                for jc in (0, 1):
                    nc.tensor.matmul(
                        pm[:, ds(512 * jc, 512)],
                        lhsT=v_sb[qa][:, b, :, h, :],
                        rhs=eb[:, :, ds(512 * jc, 512)],
                        start=(b == 0), stop=(b == 7), perf_mode=PM,
                    )

            def drain(u, hh, pm, act=False):
                if hh == 0:
                    rep_tiles[u] = srow.tile([128, NH], F32, name="repn", tag="repn")
                den_sb = srow.tile([1, NH], F32, name="den_sb", tag="den_sb")
                if act:
                    nc.scalar.activation(den_sb[:], pm[64:65, :], AF.Identity)
                else:
                    nc.vector.tensor_scalar(
                        out=den_sb[:], in0=pm[64:65, :],
                        scalar1=1.0, scalar2=None, op0=ALU.mult)
                rcp = srow.tile([1, NH], F32, name="rcph", tag="rcph")
                nc.vector.reciprocal_approx_fast(rcp[:], den_sb[:])
                di = 2 * u + hh
                nc.sync.dma_start(out=dscr_r[di:di + 1, :], in_=rcp[:])
                nc.sync.dma_start(
                    out=rep_tiles[u][ds(64 * hh, 64), :],
                    in_=dscr_r[di, :].unsqueeze(0).broadcast_to((64, NH)))
                if hh == 0:
                    nc.vector.tensor_scalar(
                        out=munn[u][0:64, :], in0=pm[0:64, :],
                        scalar1=1.0, scalar2=None, op0=ALU.mult)
                else:
                    mstg = srow.tile([64, NH], BF16, name="mstg", tag="mstg")
                    if act:
                        nc.scalar.activation(mstg[:], pm[0:64, :], AF.Identity)
                    else:
                        nc.vector.tensor_scalar(
                            out=mstg[:], in0=pm[0:64, :],
                            scalar1=1.0, scalar2=None, op0=ALU.mult)
                    nc.sync.dma_start(out=munn[u][ds(64, 64), :], in_=mstg[:])

            def head_full(u, hh):
                pas, p = UNITS[u]
                qa = 0 if pas == 0 else 1
                h = 2 * p + hh
                pm = P["pm"].tile([128, NH], F32, name="pm", tag="pm")
                qa_ap = qkT[qa][ds(32 * h, 32), :, :]
                qb_ap = qkT[1 - qa][ds(32 * h, 32), :, :]
                for b in range(8):
                    eb = epool.tile([128, 2, NH], FP8, name="eb", tag="eb")
                    for t2 in (0, 1):
                        ib = 2 * b + t2
                        sp = P["psim"].tile([128, NH], F32)
                        for jc in (0, 1):
                            nc.tensor.matmul(
                                sp[:, ds(512 * jc, 512)],
                                lhsT=qa_ap[:, :, ds(128 * ib, 128)],
                                rhs=qb_ap[:, :, ds(512 * jc, 512)],
                                start=True, stop=True, perf_mode=PM,
                                tile_position=(32 * h, 0),
                            )
                        emit_E(u, b, t2, sp, eb)
                    av_one(u, hh, b, eb, pm)
                drain(u, hh, pm)

            def head_deferred_av(u, hh):
                pm = P["pm"].tile([128, NH], F32, name="pm", tag="pm")
                ebs = sims_exps(u, hh)
                return pm, ebs

            def avs(u, hh, pm, ebs):
                for b in range(8):
                    av_one(u, hh, b, ebs[b], pm)
                drain(u, hh, pm)

            def norm_unit(u):
                pas, p = UNITS[u]
                s = 1 - pas
                nc.vector.tensor_tensor(
                    out=mT[s][:, p, :], in0=munn[u][:], in1=rep_tiles[u][:], op=ALU.mult)

            def wo_mproj(s, pool=None, act=False):
                pool = pool or P["ovl"]
                for m in (0, 1):
                    ps = pool.tile([128, NH], F32, name="ovl", tag="ovl")
                    for jc in (0, 1):
                        nc.tensor.matmul(
                            ps[:, ds(512 * jc, 512)],
                            lhsT=wop_sb[:, :, ds(128 * m, 128)],
                            rhs=mT[s][:, :, ds(512 * jc, 512)],
                            start=True, stop=True, perf_mode=PM,
                        )
                    if act:
                        nc.scalar.activation(
                            mproj[s][m][:], ps[:], AF.Identity, bias=bo_sb[m][:])
                    else:
                        nc.vector.tensor_scalar(
                            out=mproj[s][m][:], in0=ps[:],
                            scalar1=bo_sb[m][:], scalar2=None, op0=ALU.add)

            def ffn_w1_pre(t, m, pool):
                ck2 = [xTh_sb[t][0][:], xTh_sb[t][1][:]]
                ps = pool.tile([128, NH], F32, name="ovl", tag="ovl")
                for k in (0, 1):
                    for jc in (0, 1):
                        nc.tensor.matmul(
                            ps[:, ds(512 * jc, 512)],
                            lhsT=w1_sb[k][:, ds(128 * m, 128)],
                            rhs=ck2[k][:, ds(512 * jc, 512)],
                            start=(k == 0), stop=False,
                        )
                return ps

            def ffn_w1_post(t, m, ps, spool):
                ck2 = [mproj[t][0][:, :], mproj[t][1][:, :]]
                for k in (0, 1):
                    for jc in (0, 1):
                        nc.tensor.matmul(
                            ps[:, ds(512 * jc, 512)],
                            lhsT=w1_sb[2 + k][:, ds(128 * m, 128)],
                            rhs=ck2[k][:, ds(512 * jc, 512)],
                            start=False, stop=(k == 1),
                        )
                nc.vector.tensor_scalar(
                    out=h_sb[t][m][:], in0=ps[:],
                    scalar1=b1_sb[m][:], scalar2=None, op0=ALU.add)
                hq = hsqp.tile([128, NH], BF16, name=f"hsq{t}{m}", tag=f"hsq{m}")
                nc.vector.tensor_tensor(
                    out=hq[:], in0=h_sb[t][m][:], in1=h_sb[t][m][:], op=ALU.mult)
                hsq_tiles[t][m] = hq
                if m == 0:
                    stats_ps[t] = (
                        spool.tile([128, NH], F32, name="sums", tag="sums"),
                        spool.tile([128, NH], F32, name="ssqs", tag="ssqs"),
                    )
                su, sq = stats_ps[t]
                for jc in (0, 1):
                    nc.tensor.matmul(
                        su[0:1, ds(512 * jc, 512)],
                        lhsT=ones_sb[:], rhs=h_sb[t][m][:, ds(512 * jc, 512)],
                        start=(m == 0), stop=(m == 3),
                    )
                for jc in (0, 1):
                    nc.tensor.matmul(
                        sq[0:1, ds(512 * jc, 512)],
                        lhsT=ones_sb[:], rhs=hq[:, ds(512 * jc, 512)],
                        start=(m == 0), stop=(m == 3),
                    )

            def ffn_w1(t, chunks, pool=None, spool=None):
                pool = pool or P["ovl"]
                ck = [xTh_sb[t][0][:], xTh_sb[t][1][:],
                      mproj[t][0][:, :], mproj[t][1][:, :]]
                for m in chunks:
                    ps = pool.tile([128, NH], F32, name="ovl", tag="ovl")
                    for k in range(4):
                        for jc in (0, 1):
                            nc.tensor.matmul(
                                ps[:, ds(512 * jc, 512)],
                                lhsT=w1_sb[k][:, ds(128 * m, 128)],
                                rhs=ck[k][:, ds(512 * jc, 512)],
                                start=(k == 0), stop=(k == 3),
                            )
                    nc.vector.tensor_scalar(
                        out=h_sb[t][m][:], in0=ps[:],
                        scalar1=b1_sb[m][:], scalar2=None, op0=ALU.add)
                    hq = hsqp.tile([128, NH], BF16, name=f"hsq{t}{m}", tag=f"hsq{m}")
                    nc.vector.tensor_tensor(
                        out=hq[:], in0=h_sb[t][m][:], in1=h_sb[t][m][:], op=ALU.mult)
                    hsq_tiles[t][m] = hq
                    if spool is not None:
                        if m == 0:
                            stats_ps[t] = (
                                spool.tile([128, NH], F32, name="sums", tag="sums"),
                                spool.tile([128, NH], F32, name="ssqs", tag="ssqs"),
                            )
                        su, sq = stats_ps[t]
                        for jc in (0, 1):
                            nc.tensor.matmul(
                                su[0:1, ds(512 * jc, 512)],
                                lhsT=ones_sb[:], rhs=h_sb[t][m][:, ds(512 * jc, 512)],
                                start=(m == 0), stop=(m == 3),
                            )
                        for jc in (0, 1):
                            nc.tensor.matmul(
                                sq[0:1, ds(512 * jc, 512)],
                                lhsT=ones_sb[:], rhs=hq[:, ds(512 * jc, 512)],
                                start=(m == 0), stop=(m == 3),
                            )

            def ffn_stats(t, pool=None, spool=None):
                pool = pool or P["ovl"]
                if spool is not None:
                    sum_ps, ssq_ps = stats_ps[t]
                else:
                    sum_ps = pool.tile([128, NH], F32, name="ovl", tag="ovl")
                    for k in range(4):
                        for jc in (0, 1):
                            nc.tensor.matmul(
                                sum_ps[0:1, ds(512 * jc, 512)],
                                lhsT=ones_sb[:], rhs=h_sb[t][k][:, ds(512 * jc, 512)],
                                start=(k == 0), stop=(k == 3),
                            )
                nc.vector.tensor_scalar(
                    out=mu_neg[t][:], in0=sum_ps[0:1, :],
                    scalar1=-1.0 / 512.0, scalar2=None, op0=ALU.mult)
                nc.gpsimd.partition_broadcast(murep[t][:], mu_neg[t][:])
                sneg = st1p.tile([1, NH], F32, name="sneg", tag="sneg")
                nc.vector.tensor_tensor(out=sneg[:], in0=sum_ps[0:1, :], in1=mu_neg[t][:], op=ALU.mult)
                if spool is None:
                    ssq_ps = pool.tile([128, NH], F32, name="ovl", tag="ovl")
                    for k in range(4):
                        for jc in (0, 1):
                            nc.tensor.matmul(
                                ssq_ps[0:1, ds(512 * jc, 512)],
                                lhsT=ones_sb[:], rhs=hsq_tiles[t][k][:, ds(512 * jc, 512)],
                                start=(k == 0), stop=(k == 3),
                            )
                uu = st1p.tile([1, NH], F32, name="uu", tag="uu")
                nc.vector.tensor_tensor(out=uu[:], in0=ssq_ps[0:1, :], in1=sneg[:], op=ALU.add)
                u2 = st1p.tile([1, NH], F32, name="u2", tag="u2")
                nc.vector.tensor_scalar(
                    out=u2[:], in0=uu[:], scalar1=1.0 / 512.0, scalar2=EPS,
                    op0=ALU.mult, op1=ALU.add)
                nc.vector.reciprocal_approx_fast(rv_t[t][:], u2[:])

            hsq_tiles = [[None] * 4, [None] * 4]

            # ---- per-side LN-apply / gelu / W2 helpers ----
            murep = [per.tile([128, NH], BF16, name=f"murep{t}", tag=f"murep{t}") for t in (0, 1)]
            rsrep = [per.tile([128, NH], BF16, name=f"rsrep{t}", tag=f"rsrep{t}") for t in (0, 1)]

            def ln_gelu_w2(t, pool, ysbp):
                nc.scalar.activation(rsig[t][:], rv_t[t][:], AF.Sqrt)
                nc.gpsimd.partition_broadcast(rsrep[t][:], rsig[t][:])
                ps_mo = [pool.tile([128, NH], F32, name="ovl", tag="ovl") for _ in (0, 1)]
                d1s = []
                for m in range(4):
                    d1 = sstp.tile([128, NH], BF16, name=f"d1{m}", tag=f"d1{m}")
                    nc.vector.tensor_tensor(
                        out=d1[:], in0=h_sb[t][m][:], in1=murep[t][:], op=ALU.add)
                    d1s.append(d1)
                for m in range(4):
                    d2 = sstp.tile([128, NH], BF16, name="d2", tag="d2")
                    nc.vector.tensor_tensor(
                        out=d2[:], in0=d1s[m][:], in1=rsrep[t][:], op=ALU.mult)
                    nc.scalar.activation(
                        t_sb[t][m][:], d2[:], AF.Gelu,
                        bias=lnb_sb[m][:], scale=lng_sb[m][:])
                    for mo in (0, 1):
                        for jc in (0, 1):
                            nc.tensor.matmul(
                                ps_mo[mo][:, ds(512 * jc, 512)],
                                lhsT=w2_sb[m][:, ds(128 * mo, 128)],
                                rhs=t_sb[t][m][:, ds(512 * jc, 512)],
                                start=(m == 0), stop=(m == 3),
                            )
                for mo in (0, 1):
                    yt = ysbp.tile([128, NH], F32, name="yt", tag="yt")
                    nc.vector.tensor_scalar(
                        out=yt[:], in0=ps_mo[mo][:], scalar1=1.0, scalar2=None, op0=ALU.mult)
                    nc.sync.dma_start(out=y[t][mo], in_=yt[:])

            def w2_store(t, pool, ysbp):
                for mo in (0, 1):
                    ps = pool.tile([128, NH], F32, name="ovl", tag="ovl")
                    for jc in (0, 1):
                        for k in range(4):
                            nc.tensor.matmul(
                                ps[:, ds(512 * jc, 512)],
                                lhsT=w2_sb[k][:, ds(128 * mo, 128)],
                                rhs=t_sb[t][k][:, ds(512 * jc, 512)],
                                start=(k == 0), stop=(k == 3),
                            )
                    yt = ysbp.tile([128, NH], F32, name="yt", tag="yt")
                    nc.vector.tensor_scalar(
                        out=yt[:], in0=ps[:], scalar1=1.0, scalar2=None, op0=ALU.mult)
                    nc.sync.dma_start(out=y[t][mo], in_=yt[:])

            # ---- emission ----
            with tc.tile_pool(name="ysb", bufs=2) as ysbp:
                with (
                    tc.tile_pool(name="psim", bufs=2, space="PSUM") as _psim,
                    tc.tile_pool(name="pmp", bufs=1, space="PSUM") as _pmp,
                    tc.tile_pool(name="ovl", bufs=1, space="PSUM") as _ovl,
                ):
                    P["psim"] = _psim
                    P["pm"] = _pmp
                    P["ovl"] = _ovl
                    qkproj()
                    pm00, ebs00 = head_deferred_av(0, 0)
                    vproj(0)
                    avs(0, 0, pm00, ebs00)
                    pm01, ebs01 = head_deferred_av(0, 1)
                    vproj(1)
                    avs(0, 1, pm01, ebs01)
                    head_full(1, 0)
                    head_full(1, 1)
                    norm_unit(0)
                    norm_unit(1)
                    wo_mproj(1)
                    head_full(2, 0)
                    ffn_w1(1, (0,))
                    head_full(2, 1)
                    ffn_w1(1, (1, 2))
                    head_full(3, 0)
                    ffn_w1(1, (3,))
                    ffn_stats(1)
                    norm_unit(2)
                    head_full(3, 1)
                # attention PSUM freed: 8 banks for the tail
                norm_unit(3)
                with (
                    tc.tile_pool(name="post", bufs=2, space="PSUM") as postp,
                    tc.tile_pool(name="pst2", bufs=1, space="PSUM") as pst2,
                ):
                    pre0 = ffn_w1_pre(0, 0, postp)
                    wo_mproj(0, postp)
                    ffn_w1_post(0, 0, pre0, pst2)
                    ffn_w1(0, (1, 2, 3), postp, pst2)
                    ln_gelu_w2(1, postp, ysbp)
                    ffn_stats(0, postp, pst2)
                    ln_gelu_w2(0, postp, ysbp)
    return nc


_CACHE = {}


def _get_program():
    if "nc" not in _CACHE:
        nc = bacc.Bacc()
        _build(nc)
        nc.finalize()
        _CACHE["nc"] = nc
    return _CACHE["nc"]


def _bf16(a):
    return np.ascontiguousarray(a.astype(ml_dtypes.bfloat16))


def _fp8(a):
    return np.ascontiguousarray(a.astype(ml_dtypes.float8_e4m3))


def _f32(a):
    return np.ascontiguousarray(a.astype(np.float32))


def kernel(x0, x1, Wqk, bqk, Wv, bv, Wo, bo, W1, b1, ln_g, ln_b, W2, b2):
    x0, x1 = np.asarray(x0, np.float32), np.asarray(x1, np.float32)
    Wqk = np.asarray(Wqk, np.float32) * SS
    bqk = np.asarray(bqk, np.float32) * SS
    Wv = np.asarray(Wv, np.float32)
    bv = np.asarray(bv, np.float32)
    Wo = np.asarray(Wo, np.float32)
    bo = np.asarray(bo, np.float32)
    W1 = np.asarray(W1, np.float32)
    b1 = np.asarray(b1, np.float32)
    W2 = np.asarray(W2, np.float32)
    b2 = np.asarray(b2, np.float32)
    ln_g = np.asarray(ln_g, np.float32)
    ln_b = np.asarray(ln_b, np.float32)

    pidx = np.arange(128)
    col = lambda t: 64 * (pidx // 32) + 32 * t + (pidx % 32)
    # wqkp[t][i, kt, p] = Wqk[128*kt + i, col(p, t)]  (fp8-DR pair over feat chunks)
    wqkp = np.empty((2, 128, 2, 128), np.float32)
    bqkp = np.empty((2, 128, 1), np.float32)
    for t in (0, 1):
        c = col(t)
        for kt in (0, 1):
            wqkp[t, :, kt, :] = Wqk[128 * kt:128 * (kt + 1), c]
        bqkp[t, :, 0] = bqk[c]
    # wv pair layout: wvp[i, kt, :] = Wv[128*kt + i, :]
    wvp = np.stack([Wv[0:128], Wv[128:256]], axis=1)
    wop = np.stack([Wo[0:128], Wo[128:256]], axis=1)
    bv2 = np.broadcast_to(bv.reshape(1, 1, 1, 4, DH), (128, 2, 2, 4, DH))

    shared = {
        "wqkp": _fp8(wqkp),
        "bqkp": _f32(bqkp),
        "wv": _fp8(wvp),
        "bv2": _f32(bv2),
        "wop": _fp8(wop),
        "bo": _f32(bo.reshape(2, 128, 1)),
        "w1": _bf16(W1.reshape(4, 128, 2 * D)),
        "w2": _bf16(W2.reshape(4, 128, D)),
        "b1": _f32(b1.reshape(4, 128, 1)),
        "lng": _f32(ln_g.reshape(4, 128, 1)),
        "lnb": _f32(ln_b.reshape(4, 128, 1)),
    }
    in_maps = []
    for c in range(8):
        b, half = c // 2, c % 2
        p0, p1 = x0[b], x1[b]
        if half == 1:
            p0 = np.concatenate([p0[NH:], p0[:NH]], 0)
            p1 = np.concatenate([p1[NH:], p1[:NH]], 0)
        m = dict(shared)
        # fp8 x^T in k-tile-pair layout [128, 2, N]: element (i, kt, n) = x[n, 128*kt + i]
        m["x0Tq"] = _fp8(p0.T.reshape(2, 128, N).transpose(1, 0, 2))
        m["x1Tq"] = _fp8(p1.T.reshape(2, 128, N).transpose(1, 0, 2))
        m["x0Th"] = _bf16(p0[:NH].T.reshape(2, 128, NH))
        m["x1Th"] = _bf16(p1[:NH].T.reshape(2, 128, NH))
        in_maps.append(m)

    nc = _get_program()
    res = run_bass_kernel_spmd(nc, in_maps, list(range(8)))
    out0 = x0 + b2
    out1 = x1 + b2
    for c in range(8):
        b, half = c // 2, c % 2
        out0[b, half * NH:(half + 1) * NH] += res.results[c]["y0"].reshape(D, NH).T
        out1[b, half * NH:(half + 1) * NH] += res.results[c]["y1"].reshape(D, NH).T
    return out0, out1


# revision 4
# speedup vs baseline: 8799.8228x; 1.0024x over previous
"""CrossBlock Trainium2 kernel v3: 8-core SPMD, fp8 DoubleRow attention, fused
softmax denominators, FFN overlapped under the exp stream.

Sharding: core c -> (batch c//2, seq-half c%2) via host token rotation.
Per (pass, head-pair, head): sim fp8-DR (k-tiles = dh halves) -> exp (ACT) ->
AV fp8-DR with v padded to M=128 whose col 64 is ones, so PSUM row 64
accumulates the softmax denominator in the same matmuls. Normalization is
deferred: unnormalized m^T drains to SBUF, denominator rows go to DRAM, one
batched DVE reciprocal per side + broadcast + multiply produce mT fp8.
ACT runs only: Exp (attention), Sqrt (LN rsig), Gelu (tail) = 3 table loads.
Wo / W2 are fp8-DR; W1/stats run under the attention exp stream in 2 spare
PSUM banks. Copies: qkT on DVE, v-scatter + den rows on GpSimd (Pool).
"""

import numpy as np
import ml_dtypes

import concourse.bacc as bacc
import concourse.mybir as mybir
import concourse.tile as tile
from concourse.bass import ds
from concourse.bass_utils import run_bass_kernel_spmd

B, N, D, H, DH = 4, 2048, 256, 4, 64
NH = N // 2
SS = float(DH) ** -0.25
EPS = 1e-5
F32 = mybir.dt.float32
BF16 = mybir.dt.bfloat16
FP8 = mybir.dt.float8e4
AF = mybir.ActivationFunctionType
ALU = mybir.AluOpType
PM = mybir.MatmulPerfMode.DoubleRow
UNITS = [(0, 0), (0, 1), (1, 0), (1, 1)]  # (pass, head-pair); qb side = 1-pass


def _build(nc):
    xTq = [nc.dram_tensor(f"x{t}Tq", [128, 2, N], FP8, kind="ExternalInput") for t in (0, 1)]
    xTh = [nc.dram_tensor(f"x{t}Th", [2, 128, NH], BF16, kind="ExternalInput") for t in (0, 1)]
    wqkp = nc.dram_tensor("wqkp", [2, 128, 2, 128], FP8, kind="ExternalInput")
    bqkp = nc.dram_tensor("bqkp", [2, 128, 1], F32, kind="ExternalInput")
    wv = nc.dram_tensor("wv", [128, 2, D], FP8, kind="ExternalInput")
    bv2 = nc.dram_tensor("bv2", [128, 2, 2, 4, DH], F32, kind="ExternalInput")
    wop = nc.dram_tensor("wop", [128, 2, D], FP8, kind="ExternalInput")
    bo = nc.dram_tensor("bo", [2, 128, 1], F32, kind="ExternalInput")
    w1 = nc.dram_tensor("w1", [4, 128, 2 * D], BF16, kind="ExternalInput")
    w2 = nc.dram_tensor("w2", [4, 128, D], BF16, kind="ExternalInput")
    b1 = nc.dram_tensor("b1", [4, 128, 1], F32, kind="ExternalInput")
    lng = nc.dram_tensor("lng", [4, 128, 1], F32, kind="ExternalInput")
    lnb = nc.dram_tensor("lnb", [4, 128, 1], F32, kind="ExternalInput")
    y = [nc.dram_tensor(f"y{t}", [2, 128, NH], F32, kind="ExternalOutput") for t in (0, 1)]
    dscr = nc.dram_tensor("dscr", [8, NH], F32)
    dscr_r = nc.dram_tensor("dscr_r", [8, NH], F32)
    dscr2 = nc.dram_tensor("dscr2", [4, NH], BF16)

    with tile.TileContext(nc) as tc:
        with (
            tc.tile_pool(name="per", bufs=1) as per,
            tc.tile_pool(name="epool", bufs=9) as epool,
            tc.tile_pool(name="srow", bufs=2) as srow,
            tc.tile_pool(name="sst", bufs=1) as sstp,
            tc.tile_pool(name="st1", bufs=1) as st1p,
            tc.tile_pool(name="hsqp", bufs=1) as hsqp,
        ):
            # ---- loads ----
            wqkp_sb = [per.tile([128, 2, 128], FP8, name=f"wqkp{t}", tag=f"wqkp{t}") for t in (0, 1)]
            bqkp_sb = [per.tile([128, 1], F32, name=f"bqkp{t}", tag=f"bqkp{t}") for t in (0, 1)]
            for t in (0, 1):
                nc.sync.dma_start(out=bqkp_sb[t][:], in_=bqkp[t])
                nc.sync.dma_start(out=wqkp_sb[t][:], in_=wqkp[t])
            wv_sb = per.tile([128, 2, D], FP8, tag="wv")
            nc.sync.dma_start(out=wv_sb[:], in_=wv[:])
            xTq_sb = [per.tile([128, 2, N], FP8, name=f"x{t}Tq", tag=f"x{t}Tq") for t in (0, 1)]
            for t in (0, 1):
                nc.sync.dma_start(out=xTq_sb[t][:], in_=xTq[t][:])
            bv4_sb = per.tile([128, 2, 2, 4, DH], F32, tag="bv4")
            nc.sync.dma_start(out=bv4_sb[:], in_=bv2[:])
            wop_sb = per.tile([128, 2, D], FP8, tag="wop")
            nc.sync.dma_start(out=wop_sb[:], in_=wop[:])
            bo_sb = [per.tile([128, 1], F32, name=f"bo{m}", tag=f"bo{m}") for m in (0, 1)]
            for m in (0, 1):
                nc.sync.dma_start(out=bo_sb[m][:], in_=bo[m])
            w1_sb = [per.tile([128, 2 * D], BF16, name=f"w1{k}", tag=f"w1{k}") for k in range(4)]
            for k in range(4):
                nc.sync.dma_start(out=w1_sb[k][:], in_=w1[k])
            w2_sb = [per.tile([128, D], BF16, name=f"w2{k}", tag=f"w2{k}") for k in range(4)]
            for k in range(4):
                nc.sync.dma_start(out=w2_sb[k][:], in_=w2[k])
            xTh_sb = [[per.tile([128, NH], BF16, name=f"x{t}Th{k}", tag=f"x{t}Th{k}") for k in (0, 1)] for t in (0, 1)]
            for t in (0, 1):
                for k in (0, 1):
                    nc.sync.dma_start(out=xTh_sb[t][k][:], in_=xTh[t][k])

            b1_sb = [per.tile([128, 1], F32, name=f"b1{k}", tag=f"b1{k}") for k in range(4)]
            lng_sb = [per.tile([128, 1], F32, name=f"lng{k}", tag=f"lng{k}") for k in range(4)]
            lnb_sb = [per.tile([128, 1], F32, name=f"lnb{k}", tag=f"lnb{k}") for k in range(4)]
            for k in range(4):
                nc.sync.dma_start(out=b1_sb[k][:], in_=b1[k])
                nc.sync.dma_start(out=lng_sb[k][:], in_=lng[k])
                nc.sync.dma_start(out=lnb_sb[k][:], in_=lnb[k])
            ones_sb = per.tile([128, 1], BF16, tag="ones")
            nc.vector.memset(ones_sb[:], 1.0)

            qkT = [per.tile([128, 2, N], FP8, name=f"qkT{t}", tag=f"qkT{t}") for t in (0, 1)]
            # v: [tok128, bp8, kt2, h4, 128]; col 64 = ones (den), cols 65+ unused
            v_sb = [per.tile([128, 8, 2, 4, 128], FP8, name=f"v{t}", tag=f"v{t}") for t in (0, 1)]
            for s in (0, 1):
                nc.gpsimd.memset(v_sb[s][:, :, :, :, 64:65], 1.0)
            mT = [per.tile([128, 2, NH], FP8, name=f"mT{t}", tag=f"mT{t}") for t in (0, 1)]
            munn = [per.tile([128, NH], BF16, name=f"munn{u}", tag=f"munn{u}") for u in range(4)]
            mproj = [[per.tile([128, NH], BF16, name=f"mp{t}{m}", tag=f"mp{t}{m}") for m in (0, 1)] for t in (0, 1)]
            h_sb = [[per.tile([128, NH], BF16, name=f"h{t}{m}", tag=f"h{m}") for m in range(4)] for t in (0, 1)]
            mu_neg = [per.tile([1, NH], BF16, name=f"mu{t}", tag=f"mu{t}") for t in (0, 1)]
            rsig = [per.tile([1, NH], BF16, name=f"rs{t}", tag=f"rs{t}") for t in (0, 1)]
            rv_t = [per.tile([1, NH], F32, name=f"rv{t}", tag=f"rv{t}") for t in (0, 1)]
            t_sb = [[per.tile([128, NH], BF16, name=f"g{t}{m}", tag=f"g{m}") for m in range(4)] for t in (0, 1)]

            P = {}
            rep_tiles = [None] * 4
            stats_ps = [None, None]
            # ---- emission helpers ----
            def qkproj():
                for s in (0, 1):
                    for half in (0, 1):
                        for t in (0, 1):
                            sp = P["psim"].tile([128, NH], F32)
                            for jj in (0, 1):
                                nc.tensor.matmul(
                                    sp[:, ds(512 * jj, 512)],
                                    lhsT=wqkp_sb[t][:],
                                    rhs=xTq_sb[s][:, :, ds(1024 * half + 512 * jj, 512)],
                                    start=True, stop=True, perf_mode=PM,
                                )
                            if t == 0:
                                nc.scalar.activation(
                                    qkT[s][:, t, ds(NH * half, NH)], sp[:],
                                    AF.Identity, bias=bqkp_sb[t][:])
                            else:
                                nc.vector.tensor_scalar(
                                    out=qkT[s][:, t, ds(NH * half, NH)],
                                    in0=sp[:],
                                    scalar1=bqkp_sb[t][:], scalar2=None, op0=ALU.add)

            def vproj(s):
                for bq in range(4):  # 2 block-pairs per psum tile
                    ps = P["ovl"].tile([128, NH], F32, name="ovl", tag="ovl")
                    for q in range(4):  # 4 token-blocks of 128
                        ib = 4 * bq + q
                        nc.tensor.matmul(
                            ps[:, ds(256 * q, 256)],
                            lhsT=xTq_sb[s][:, :, ds(128 * ib, 128)],
                            rhs=wv_sb[:],
                            start=True, stop=True, perf_mode=PM,
                        )
                    nc.vector.tensor_tensor(
                        out=v_sb[s][:, ds(2 * bq, 2), :, :, 0:DH],
                        in0=ps[:, :].rearrange("p (b t h c) -> p b t h c", b=2, t=2, c=DH),
                        in1=bv4_sb[:],
                        op=ALU.add)

            POLY_PAIRS = ()

            def emit_E(u, b, t2, sp, eb):
                if u > 0 and b in POLY_PAIRS:
                    c = sstp.tile([128, NH], BF16, name="px", tag="px")
                    nc.vector.tensor_scalar(
                        out=c[:], in0=sp[:], scalar1=1.0, scalar2=None, op0=ALU.mult)
                    v = sstp.tile([128, NH], BF16, name="py", tag="py")
                    nc.vector.tensor_scalar(
                        out=v[:], in0=c[:], scalar1=0.5, scalar2=1.0,
                        op0=ALU.mult, op1=ALU.add)
                    w = sstp.tile([128, NH], BF16, name="pw", tag="pz")
                    nc.vector.tensor_tensor(out=w[:], in0=v[:], in1=c[:], op=ALU.mult)
                    nc.vector.tensor_scalar(
                        out=eb[:, t2, :], in0=w[:], scalar1=1.0, scalar2=None, op0=ALU.add)
                else:
                    nc.scalar.activation(eb[:, t2, :], sp[:], AF.Exp)

            def sims_exps(u, hh):
                pas, p = UNITS[u]
                qa, qb = (0, 1) if pas == 0 else (1, 0)
                h = 2 * p + hh
                qa_ap = qkT[qa][ds(32 * h, 32), :, :]
                qb_ap = qkT[qb][ds(32 * h, 32), :, :]
                ebs = []
                for b in range(8):
                    eb = epool.tile([128, 2, NH], FP8, name="eb", tag="eb")
                    ebs.append(eb)
                    for t2 in (0, 1):
                        ib = 2 * b + t2
                        sp = P["psim"].tile([128, NH], F32)
                        for jc in (0, 1):
                            nc.tensor.matmul(
                                sp[:, ds(512 * jc, 512)],
                                lhsT=qa_ap[:, :, ds(128 * ib, 128)],
                                rhs=qb_ap[:, :, ds(512 * jc, 512)],
                                start=True, stop=True, perf_mode=PM,
                                tile_position=(32 * h, 0),
                            )
                        nc.scalar.activation(eb[:, t2, :], sp[:], AF.Exp)
                return ebs

            def av_one(u, hh, b, eb, pm):
                pas, p = UNITS[u]
                qa = 0 if pas == 0 else 1
                h = 2 * p + hh
                for jc in (0, 1):
                    nc.tensor.matmul(
                        pm[:, ds(512 * jc, 512)],
                        lhsT=v_sb[qa][:, b, :, h, :],
                        rhs=eb[:, :, ds(512 * jc, 512)],
                        start=(b == 0), stop=(b == 7), perf_mode=PM,
                    )

            def drain(u, hh, pm, act=False):
                if hh == 0:
                    rep_tiles[u] = srow.tile([128, NH], F32, name="repn", tag="repn")
                den_sb = srow.tile([1, NH], F32, name="den_sb", tag="den_sb")
                if act:
                    nc.scalar.activation(den_sb[:], pm[64:65, :], AF.Identity)
                else:
                    nc.vector.tensor_scalar(
                        out=den_sb[:], in0=pm[64:65, :],
                        scalar1=1.0, scalar2=None, op0=ALU.mult)
                rcp = srow.tile([1, NH], F32, name="rcph", tag="rcph")
                nc.vector.reciprocal_approx_fast(rcp[:], den_sb[:])
                di = 2 * u + hh
                nc.sync.dma_start(out=dscr_r[di:di + 1, :], in_=rcp[:])
                nc.sync.dma_start(
                    out=rep_tiles[u][ds(64 * hh, 64), :],
                    in_=dscr_r[di, :].unsqueeze(0).broadcast_to((64, NH)))
                if hh == 0:
                    nc.vector.tensor_scalar(
                        out=munn[u][0:64, :], in0=pm[0:64, :],
                        scalar1=1.0, scalar2=None, op0=ALU.mult)
                else:
                    mstg = srow.tile([64, NH], BF16, name="mstg", tag="mstg")
                    if act:
                        nc.scalar.activation(mstg[:], pm[0:64, :], AF.Identity)
                    else:
                        nc.vector.tensor_scalar(
                            out=mstg[:], in0=pm[0:64, :],
                            scalar1=1.0, scalar2=None, op0=ALU.mult)
                    nc.sync.dma_start(out=munn[u][ds(64, 64), :], in_=mstg[:])

            def head_full(u, hh):
                pas, p = UNITS[u]
                qa = 0 if pas == 0 else 1
                h = 2 * p + hh
                pm = P["pm"].tile([128, NH], F32, name="pm", tag="pm")
                qa_ap = qkT[qa][ds(32 * h, 32), :, :]
                qb_ap = qkT[1 - qa][ds(32 * h, 32), :, :]
                for b in range(8):
                    eb = epool.tile([128, 2, NH], FP8, name="eb", tag="eb")
                    for t2 in (0, 1):
                        ib = 2 * b + t2
                        sp = P["psim"].tile([128, NH], F32)
                        for jc in (0, 1):
                            nc.tensor.matmul(
                                sp[:, ds(512 * jc, 512)],
                                lhsT=qa_ap[:, :, ds(128 * ib, 128)],
                                rhs=qb_ap[:, :, ds(512 * jc, 512)],
                                start=True, stop=True, perf_mode=PM,
                                tile_position=(32 * h, 0),
                            )
                        emit_E(u, b, t2, sp, eb)
                    av_one(u, hh, b, eb, pm)
                drain(u, hh, pm)

            def head_deferred_av(u, hh):
                pm = P["pm"].tile([128, NH], F32, name="pm", tag="pm")
                ebs = sims_exps(u, hh)
                return pm, ebs

            def avs(u, hh, pm, ebs):
                for b in range(8):
                    av_one(u, hh, b, ebs[b], pm)
                drain(u, hh, pm)

            def norm_unit(u, half=None):
                pas, p = UNITS[u]
                s = 1 - pas
                if half is None:
                    nc.vector.tensor_tensor(
                        out=mT[s][:, p, :], in0=munn[u][:], in1=rep_tiles[u][:], op=ALU.mult)
                else:
                    r = ds(64 * half, 64)
                    nc.vector.tensor_tensor(
                        out=mT[s][r, p, :], in0=munn[u][r, :], in1=rep_tiles[u][r, :], op=ALU.mult)

            def wo_mproj(s, pool=None, act=False):
                pool = pool or P["ovl"]
                for m in (0, 1):
                    ps = pool.tile([128, NH], F32, name="ovl", tag="ovl")
                    for jc in (0, 1):
                        nc.tensor.matmul(
                            ps[:, ds(512 * jc, 512)],
                            lhsT=wop_sb[:, :, ds(128 * m, 128)],
                            rhs=mT[s][:, :, ds(512 * jc, 512)],
                            start=True, stop=True, perf_mode=PM,
                        )
                    if act:
                        nc.scalar.activation(
                            mproj[s][m][:], ps[:], AF.Identity, bias=bo_sb[m][:])
                    else:
                        nc.vector.tensor_scalar(
                            out=mproj[s][m][:], in0=ps[:],
                            scalar1=bo_sb[m][:], scalar2=None, op0=ALU.add)

            def ffn_w1_pre(t, m, pool):
                ck2 = [xTh_sb[t][0][:], xTh_sb[t][1][:]]
                ps = pool.tile([128, NH], F32, name="ovl", tag="ovl")
                for k in (0, 1):
                    for jc in (0, 1):
                        nc.tensor.matmul(
                            ps[:, ds(512 * jc, 512)],
                            lhsT=w1_sb[k][:, ds(128 * m, 128)],
                            rhs=ck2[k][:, ds(512 * jc, 512)],
                            start=(k == 0), stop=False,
                        )
                return ps

            def ffn_w1_post(t, m, ps, spool):
                ck2 = [mproj[t][0][:, :], mproj[t][1][:, :]]
                for k in (0, 1):
                    for jc in (0, 1):
                        nc.tensor.matmul(
                            ps[:, ds(512 * jc, 512)],
                            lhsT=w1_sb[2 + k][:, ds(128 * m, 128)],
                            rhs=ck2[k][:, ds(512 * jc, 512)],
                            start=False, stop=(k == 1),
                        )
                nc.vector.tensor_scalar(
                    out=h_sb[t][m][:], in0=ps[:],
                    scalar1=b1_sb[m][:], scalar2=None, op0=ALU.add)
                hq = hsqp.tile([128, NH], BF16, name=f"hsq{t}{m}", tag=f"hsq{m}")
                nc.vector.tensor_tensor(
                    out=hq[:], in0=h_sb[t][m][:], in1=h_sb[t][m][:], op=ALU.mult)
                hsq_tiles[t][m] = hq
                if m == 0:
                    stats_ps[t] = (
                        spool.tile([128, NH], F32, name="sums", tag="sums"),
                        spool.tile([128, NH], F32, name="ssqs", tag="ssqs"),
                    )
                su, sq = stats_ps[t]
                for jc in (0, 1):
                    nc.tensor.matmul(
                        su[0:1, ds(512 * jc, 512)],
                        lhsT=ones_sb[:], rhs=h_sb[t][m][:, ds(512 * jc, 512)],
                        start=(m == 0), stop=(m == 3),
                    )
                for jc in (0, 1):
                    nc.tensor.matmul(
                        sq[0:1, ds(512 * jc, 512)],
                        lhsT=ones_sb[:], rhs=hq[:, ds(512 * jc, 512)],
                        start=(m == 0), stop=(m == 3),
                    )

            def ffn_w1(t, chunks, pool=None, spool=None):
                pool = pool or P["ovl"]
                ck = [xTh_sb[t][0][:], xTh_sb[t][1][:],
                      mproj[t][0][:, :], mproj[t][1][:, :]]
                for m in chunks:
                    ps = pool.tile([128, NH], F32, name="ovl", tag="ovl")
                    for k in range(4):
                        for jc in (0, 1):
                            nc.tensor.matmul(
                                ps[:, ds(512 * jc, 512)],
                                lhsT=w1_sb[k][:, ds(128 * m, 128)],
                                rhs=ck[k][:, ds(512 * jc, 512)],
                                start=(k == 0), stop=(k == 3),
                            )
                    nc.vector.tensor_scalar(
                        out=h_sb[t][m][:], in0=ps[:],
                        scalar1=b1_sb[m][:], scalar2=None, op0=ALU.add)
                    hq = hsqp.tile([128, NH], BF16, name=f"hsq{t}{m}", tag=f"hsq{m}")
                    nc.vector.tensor_tensor(
                        out=hq[:], in0=h_sb[t][m][:], in1=h_sb[t][m][:], op=ALU.mult)
                    hsq_tiles[t][m] = hq
                    if spool is not None:
                        if m == 0:
                            stats_ps[t] = (
                                spool.tile([128, NH], F32, name="sums", tag="sums"),
                                spool.tile([128, NH], F32, name="ssqs", tag="ssqs"),
                            )
                        su, sq = stats_ps[t]
                        for jc in (0, 1):
                            nc.tensor.matmul(
                                su[0:1, ds(512 * jc, 512)],
                                lhsT=ones_sb[:], rhs=h_sb[t][m][:, ds(512 * jc, 512)],
                                start=(m == 0), stop=(m == 3),
                            )
                        for jc in (0, 1):
                            nc.tensor.matmul(
                                sq[0:1, ds(512 * jc, 512)],
                                lhsT=ones_sb[:], rhs=hq[:, ds(512 * jc, 512)],
                                start=(m == 0), stop=(m == 3),
                            )

            def ffn_stats(t, pool=None, spool=None):
                pool = pool or P["ovl"]
                if spool is not None:
                    sum_ps, ssq_ps = stats_ps[t]
                else:
                    sum_ps = pool.tile([128, NH], F32, name="ovl", tag="ovl")
                    for k in range(4):
                        for jc in (0, 1):
                            nc.tensor.matmul(
                                sum_ps[0:1, ds(512 * jc, 512)],
                                lhsT=ones_sb[:], rhs=h_sb[t][k][:, ds(512 * jc, 512)],
                                start=(k == 0), stop=(k == 3),
                            )
                nc.vector.tensor_scalar(
                    out=mu_neg[t][:], in0=sum_ps[0:1, :],
                    scalar1=-1.0 / 512.0, scalar2=None, op0=ALU.mult)
                nc.gpsimd.partition_broadcast(murep[t][:], mu_neg[t][:])
                sneg = st1p.tile([1, NH], F32, name="sneg", tag="sneg")
                nc.vector.tensor_tensor(out=sneg[:], in0=sum_ps[0:1, :], in1=mu_neg[t][:], op=ALU.mult)
                if spool is None:
                    ssq_ps = pool.tile([128, NH], F32, name="ovl", tag="ovl")
                    for k in range(4):
                        for jc in (0, 1):
                            nc.tensor.matmul(
                                ssq_ps[0:1, ds(512 * jc, 512)],
                                lhsT=ones_sb[:], rhs=hsq_tiles[t][k][:, ds(512 * jc, 512)],
                                start=(k == 0), stop=(k == 3),
                            )
                uu = st1p.tile([1, NH], F32, name="uu", tag="uu")
                nc.vector.tensor_tensor(out=uu[:], in0=ssq_ps[0:1, :], in1=sneg[:], op=ALU.add)
                u2 = st1p.tile([1, NH], F32, name="u2", tag="u2")
                nc.vector.tensor_scalar(
                    out=u2[:], in0=uu[:], scalar1=1.0 / 512.0, scalar2=EPS,
                    op0=ALU.mult, op1=ALU.add)
                nc.vector.reciprocal_approx_fast(rv_t[t][:], u2[:])

            hsq_tiles = [[None] * 4, [None] * 4]

            # ---- per-side LN-apply / gelu / W2 helpers ----
            murep = [per.tile([128, NH], BF16, name=f"murep{t}", tag=f"murep{t}") for t in (0, 1)]
            rsrep = [per.tile([128, NH], BF16, name=f"rsrep{t}", tag=f"rsrep{t}") for t in (0, 1)]

            def ln_gelu_w2(t, pool, ysbp):
                nc.scalar.activation(rsig[t][:], rv_t[t][:], AF.Sqrt)
                nc.gpsimd.partition_broadcast(rsrep[t][:], rsig[t][:])
                ps_mo = [pool.tile([128, NH], F32, name="ovl", tag="ovl") for _ in (0, 1)]
                d1s = []
                for m in range(4):
                    d1 = sstp.tile([128, NH], BF16, name=f"d1{m}", tag=f"d1{m}")
                    nc.vector.tensor_tensor(
                        out=d1[:], in0=h_sb[t][m][:], in1=murep[t][:], op=ALU.add)
                    d1s.append(d1)
                for m in range(4):
                    d2 = sstp.tile([128, NH], BF16, name="d2", tag="d2")
                    nc.vector.tensor_tensor(
                        out=d2[:], in0=d1s[m][:], in1=rsrep[t][:], op=ALU.mult)
                    nc.scalar.activation(
                        t_sb[t][m][:], d2[:], AF.Gelu,
                        bias=lnb_sb[m][:], scale=lng_sb[m][:])
                    for mo in (0, 1):
                        for jc in (0, 1):
                            nc.tensor.matmul(
                                ps_mo[mo][:, ds(512 * jc, 512)],
                                lhsT=w2_sb[m][:, ds(128 * mo, 128)],
                                rhs=t_sb[t][m][:, ds(512 * jc, 512)],
                                start=(m == 0), stop=(m == 3),
                            )
                for mo in (0, 1):
                    yt = ysbp.tile([128, NH], F32, name="yt", tag="yt")
                    for jh in (0, 1):
                        nc.vector.tensor_scalar(
                            out=yt[:, ds(512 * jh, 512)], in0=ps_mo[mo][:, ds(512 * jh, 512)],
                            scalar1=1.0, scalar2=None, op0=ALU.mult)
                        nc.sync.dma_start(out=y[t][mo, :, ds(512 * jh, 512)], in_=yt[:, ds(512 * jh, 512)])

            def w2_store(t, pool, ysbp):
                for mo in (0, 1):
                    ps = pool.tile([128, NH], F32, name="ovl", tag="ovl")
                    for jc in (0, 1):
                        for k in range(4):
                            nc.tensor.matmul(
                                ps[:, ds(512 * jc, 512)],
                                lhsT=w2_sb[k][:, ds(128 * mo, 128)],
                                rhs=t_sb[t][k][:, ds(512 * jc, 512)],
                                start=(k == 0), stop=(k == 3),
                            )
                    yt = ysbp.tile([128, NH], F32, name="yt", tag="yt")
                    nc.vector.tensor_scalar(
                        out=yt[:], in0=ps[:], scalar1=1.0, scalar2=None, op0=ALU.mult)
                    nc.sync.dma_start(out=y[t][mo], in_=yt[:])

            # ---- emission ----
            with tc.tile_pool(name="ysb", bufs=2) as ysbp:
                with (
                    tc.tile_pool(name="psim", bufs=2, space="PSUM") as _psim,
                    tc.tile_pool(name="pmp", bufs=1, space="PSUM") as _pmp,
                    tc.tile_pool(name="ovl", bufs=1, space="PSUM") as _ovl,
                ):
                    P["psim"] = _psim
                    P["pm"] = _pmp
                    P["ovl"] = _ovl
                    qkproj()
                    pm00, ebs00 = head_deferred_av(0, 0)
                    vproj(0)
                    avs(0, 0, pm00, ebs00)
                    pm01, ebs01 = head_deferred_av(0, 1)
                    vproj(1)
                    avs(0, 1, pm01, ebs01)
                    head_full(1, 0)
                    head_full(1, 1)
                    norm_unit(0)
                    norm_unit(1)
                    wo_mproj(1)
                    head_full(2, 0)
                    ffn_w1(1, (0,))
                    head_full(2, 1)
                    ffn_w1(1, (1, 2))
                    head_full(3, 0)
                    ffn_w1(1, (3,))
                    ffn_stats(1)
                    norm_unit(2)
                    norm_unit(3, half=0)
                    head_full(3, 1)
                # attention PSUM freed: 8 banks for the tail
                norm_unit(3, half=1)
                with (
                    tc.tile_pool(name="post", bufs=2, space="PSUM") as postp,
                    tc.tile_pool(name="pst2", bufs=1, space="PSUM") as pst2,
                ):
                    pre0 = ffn_w1_pre(0, 0, postp)
                    wo_mproj(0, postp)
                    ffn_w1_post(0, 0, pre0, pst2)
                    ffn_w1(0, (1, 2, 3), postp, pst2)
                    ln_gelu_w2(1, postp, ysbp)
                    ffn_stats(0, postp, pst2)
                    ln_gelu_w2(0, postp, ysbp)
    return nc


_CACHE = {}


def _get_program():
    if "nc" not in _CACHE:
        nc = bacc.Bacc()
        _build(nc)
        nc.finalize()
        _CACHE["nc"] = nc
    return _CACHE["nc"]


def _bf16(a):
    return np.ascontiguousarray(a.astype(ml_dtypes.bfloat16))


def _fp8(a):
    return np.ascontiguousarray(a.astype(ml_dtypes.float8_e4m3))


def _f32(a):
    return np.ascontiguousarray(a.astype(np.float32))


def kernel(x0, x1, Wqk, bqk, Wv, bv, Wo, bo, W1, b1, ln_g, ln_b, W2, b2):
    x0, x1 = np.asarray(x0, np.float32), np.asarray(x1, np.float32)
    Wqk = np.asarray(Wqk, np.float32) * SS
    bqk = np.asarray(bqk, np.float32) * SS
    Wv = np.asarray(Wv, np.float32)
    bv = np.asarray(bv, np.float32)
    Wo = np.asarray(Wo, np.float32)
    bo = np.asarray(bo, np.float32)
    W1 = np.asarray(W1, np.float32)
    b1 = np.asarray(b1, np.float32)
    W2 = np.asarray(W2, np.float32)
    b2 = np.asarray(b2, np.float32)
    ln_g = np.asarray(ln_g, np.float32)
    ln_b = np.asarray(ln_b, np.float32)

    pidx = np.arange(128)
    col = lambda t: 64 * (pidx // 32) + 32 * t + (pidx % 32)
    # wqkp[t][i, kt, p] = Wqk[128*kt + i, col(p, t)]  (fp8-DR pair over feat chunks)
    wqkp = np.empty((2, 128, 2, 128), np.float32)
    bqkp = np.empty((2, 128, 1), np.float32)
    for t in (0, 1):
        c = col(t)
        for kt in (0, 1):
            wqkp[t, :, kt, :] = Wqk[128 * kt:128 * (kt + 1), c]
        bqkp[t, :, 0] = bqk[c]
    # wv pair layout: wvp[i, kt, :] = Wv[128*kt + i, :]
    wvp = np.stack([Wv[0:128], Wv[128:256]], axis=1)
    wop = np.stack([Wo[0:128], Wo[128:256]], axis=1)
    bv2 = np.broadcast_to(bv.reshape(1, 1, 1, 4, DH), (128, 2, 2, 4, DH))

    shared = {
        "wqkp": _fp8(wqkp),
        "bqkp": _f32(bqkp),
        "wv": _fp8(wvp),
        "bv2": _f32(bv2),
        "wop": _fp8(wop),
        "bo": _f32(bo.reshape(2, 128, 1)),
        "w1": _bf16(W1.reshape(4, 128, 2 * D)),
        "w2": _bf16(W2.reshape(4, 128, D)),
        "b1": _f32(b1.reshape(4, 128, 1)),
        "lng": _f32(ln_g.reshape(4, 128, 1)),
        "lnb": _f32(ln_b.reshape(4, 128, 1)),
    }
    in_maps = []
    for c in range(8):
        b, half = c // 2, c % 2
        p0, p1 = x0[b], x1[b]
        if half == 1:
            p0 = np.concatenate([p0[NH:], p0[:NH]], 0)
            p1 = np.concatenate([p1[NH:], p1[:NH]], 0)
        m = dict(shared)
        # fp8 x^T in k-tile-pair layout [128, 2, N]: element (i, kt, n) = x[n, 128*kt + i]
        m["x0Tq"] = _fp8(p0.T.reshape(2, 128, N).transpose(1, 0, 2))
        m["x1Tq"] = _fp8(p1.T.reshape(2, 128, N).transpose(1, 0, 2))
        m["x0Th"] = _bf16(p0[:NH].T.reshape(2, 128, NH))
        m["x1Th"] = _bf16(p1[:NH].T.reshape(2, 128, NH))
        in_maps.append(m)

    nc = _get_program()
    res = run_bass_kernel_spmd(nc, in_maps, list(range(8)))
    out0 = x0 + b2
    out1 = x1 + b2
    for c in range(8):
        b, half = c // 2, c % 2
        out0[b, half * NH:(half + 1) * NH] += res.results[c]["y0"].reshape(D, NH).T
        out1[b, half * NH:(half + 1) * NH] += res.results[c]["y1"].reshape(D, NH).T
    return out0, out1
